# revision 12
# baseline (speedup 1.0000x reference)
"""Trainium2 Bass kernel for nn_Biaffine (B=4, S=512, D=512, R=64).

Math: the reference computes
    left = einsum('bxi,irj,byj->bxyr', hf, U1, hb)
    out  = mean_y(left + rf[:, :, None] + rb[:, None] + bias)
The mean over y commutes with everything:
    mean_y(left)[b,x,r] = sum_ij hf[b,x,i] U1[i,r,j] hbbar[b,j],
    hbbar = mean_y(hb).
So out[b,x,r] = sum_i hf[b,x,i] * (V[b,i,r] + U2a[i,r]) + rbbar[b,r] + bias[r]
with V[b,i,r] = sum_j U1[i,r,j] hbbar[b,j], rbbar = hbbar @ U2b.

Sharding: tensor-parallel over r (dep_vec_dim): core c owns r in [8c, 8c+8).
Every core loads all of hb and computes hbbar locally - no collective.
Precision ladder (the 2e-2 rel-err gate allows reduced precision; all
K-dim accumulation stays in fp32 PSUM):
  fp8 variant (default): U1 as e4m3 scaled x64 (hbbar cast to e4m3 too,
  the x64/S undone during the PSUM drain), hb as e4m3, hf as bf16 (the
  rf = hf@U2a term dominates the output so hf stays 16-bit).
  Measured rel-err 8.3e-3. bf16 variant: everything bf16, 3.0e-3.
All tiles are host-prepacked so each DMA is one contiguous run per
partition - strided APs were measured 4.4x slower than packed ones.
"""

import os
import sys

import numpy as np

try:
    import concourse.bass as bass  # noqa: F401
except ImportError:  # pragma: no cover
    sys.path.insert(0, "/opt/trn_rl_repo")

B, S, D, R = 4, 512, 512, 64
NCORES = 8
RB = R // NCORES  # 8 r's per core
P = 128
JC = D // P  # 4 j-chunks
IC = D // P  # 4 i-chunks
SY = S // NCORES  # 64 y's per core (AR variant only)

# per-partition byte offsets inside the single packed input tile: all five
# logical inputs ride one DMA (each extra dma_start costs ~1-2us of in-order
# queue completion latency per body, which dominated v4's tail)
OFF_HFT = 0                       # B*IC*S bf16 = 16384 B
OFF_U1 = OFF_HFT + B * IC * S * 2  # JC*RB*D fp8/bf16
OFF_HB = 0                        # filled in at import below
OFF_U2 = 0
OFF_BIAS = 0
TOT = 0


def _layout(lobytes):
    """Byte offsets for the packed tile; lobytes = 1 (fp8) or 2 (bf16)."""
    off_u1 = OFF_U1
    off_hb = off_u1 + JC * RB * D * lobytes
    off_u2 = off_hb + JC * B * S * lobytes
    off_bias = off_u2 + IC * 2 * RB * 4
    tot = off_bias + RB * 4
    return off_u1, off_hb, off_u2, off_bias, tot

# "fp8": U1 and hb in float8_e4m3 (default). "bf16": U1 and hb in bf16.
VARIANT = os.environ.get("BASS_KERNEL_VARIANT", "fp8")
# truncate the body after stage N (5 = full kernel); timing ablations only
STAGE = int(os.environ.get("BASS_KERNEL_STAGE", "5"))

_NC_CACHE = {}


def _build_nc(n_repeat=1, variant=None):
    import concourse.bacc as bacc
    import concourse.mybir as mybir
    import concourse.tile as tile
    from concourse.masks import make_identity

    if variant is None:
        variant = VARIANT
    fp32 = mybir.dt.float32
    bf16 = mybir.dt.bfloat16
    lodt = mybir.dt.float8e4 if variant == "fp8" else bf16

    nc = bacc.Bacc("TRN2", target_bir_lowering=False, debug=False, num_devices=NCORES)

    u8 = mybir.dt.uint8
    lobytes = 1 if variant == "fp8" else 2
    _, _, _, _, tot = _layout(lobytes)
    mega_d = nc.dram_tensor("mega", [P, tot], u8, kind="ExternalInput")
    out_d = nc.dram_tensor("out", [RB, B, S], fp32, kind="ExternalOutput")

    with tile.TileContext(nc) as tc:
        with (
            tc.tile_pool(name="const", bufs=1) as cpool,
            tc.tile_pool(name="data", bufs=1) as dpool,
            tc.tile_pool(name="psum", bufs=8, space="PSUM") as ppool,
            tc.tile_pool(name="dram", bufs=1, space="DRAM") as drpool,
        ):
            identity_sq = cpool.tile([100, 100], fp32, tag="identity_sq")
            make_identity(nc, identity_sq)
            ones1 = cpool.tile([1, S], fp32, tag="ones1")
            nc.vector.memset(ones1, 1.0)

            for _rep in range(n_repeat):
                _emit_body(
                    nc, dpool, ppool, drpool, fp32, bf16, lodt, ones1,
                    identity_sq, mega_d, out_d, variant,
                )

    nc.compile()
    return nc


def _emit_body(
    nc, dpool, ppool, drpool, fp32, bf16, lodt, ones1, identity_sq,
    mega_d, out_d, variant,
):
    import concourse.mybir as mybir

    u8 = mybir.dt.uint8
    lobytes = 1 if variant == "fp8" else 2
    off_u1, off_hb, off_u2, off_bias, tot = _layout(lobytes)

    # fp8: U1 is scaled x64 on the host (e4m3 underflows below ~2^-9) and
    # 1/S is not folded in; 64*S = 2^15 exactly, undone in the out drain
    drain_mult = 1.0 / (64.0 * S) if variant == "fp8" else 1.0

    hbbarT = dpool.tile([P, JC * B], fp32, tag="hbbarT", bufs=2)
    hbbarTq = dpool.tile([P, JC * B], lodt, tag="hbbarTq", bufs=2)
    vass = dpool.tile([P, IC, B, RB], bf16, tag="vass", bufs=2)
    out_sb = dpool.tile([RB, B, S], fp32, tag="outsb", bufs=2)

    # --- ONE input DMA: all five logical inputs, packed per partition ---
    mega = dpool.tile([P, tot], u8, tag="mega", bufs=2)
    nc.sync.dma_start(out=mega, in_=mega_d.ap())

    # typed views into the packed tile
    hftv = (
        mega[:, OFF_HFT : OFF_HFT + B * IC * S * 2]
        .bitcast(bf16)
        .rearrange("p (b ic x) -> p b ic x", b=B, ic=IC)
    )
    u1v = (
        mega[:, off_u1 : off_u1 + JC * RB * D * lobytes]
        .bitcast(lodt)
        .rearrange("p (jc r i) -> p jc r i", jc=JC, r=RB)
    )
    hbv = (
        mega[:, off_hb : off_hb + JC * B * S * lobytes]
        .bitcast(lodt)
        .rearrange("p (jc b y) -> p jc b y", jc=JC, b=B)
    )
    u2v = (
        mega[:, off_u2 : off_u2 + IC * 2 * RB * 4]
        .bitcast(fp32)
        .rearrange("p (ic c) -> p ic c", ic=IC)
    )
    biasv = mega[0:1, off_bias : off_bias + RB * 4].bitcast(fp32)

    if STAGE < 1:
        nc.vector.memset(out_sb[:, :1, :1], 0.0)
        nc.scalar.dma_start(out=out_d.ap(), in_=out_sb)
        return

    # hbbarT[j, b] = sum_y hb[b, y, j], one 3D-AP reduce per jc
    for jc in range(JC):
        nc.vector.reduce_sum(
            hbbarT[:, jc * B : (jc + 1) * B, None],
            hbv[:, jc, :, :],
            axis=mybir.AxisListType.X,
        )

    # low-precision copy of hbbar for the V matmuls (matches U1's dtype)
    nc.vector.tensor_copy(out=hbbarTq, in_=hbbarT)

    if STAGE < 2:
        nc.vector.memset(out_sb[:, :1, :1], 0.0)
        nc.scalar.dma_start(out=out_d.ap(), in_=out_sb)
        return

    # --- rbbT[r, b] = (hbbar @ U2b + bias)^T computed directly transposed:
    # stationary U2b [j128, RB], moving hbbar [j128, B]; bias via K=1 matmul
    ps_rbt = ppool.tile([P, 512], fp32, tag="ps")
    for jc in range(JC):
        nc.tensor.matmul(
            ps_rbt[:RB, :B],
            u2v[:, jc, RB : 2 * RB],
            hbbarT[:, jc * B : (jc + 1) * B],
            start=(jc == 0),
            stop=False,
        )
    nc.tensor.matmul(
        ps_rbt[:RB, :B], biasv, ones1[:1, :B], start=False, stop=True
    )
    rbbT = dpool.tile([RB, B], fp32, tag="rbbT", bufs=2)
    nc.vector.tensor_copy(out=rbbT, in_=ps_rbt[:RB, :B])

    if STAGE < 3:
        nc.vector.memset(out_sb[:, :1, :1], 0.0)
        nc.scalar.dma_start(out=out_d.ap(), in_=out_sb)
        return

    # --- V[b, i] per r: hbbarTq stationary (LDW = 4 cols), U1 streams as
    # the N=512 moving operand; 4 r's share each PSUM tile at partition
    # offsets {0,32,64,96} so only 8 [b,i]->[i,b] PE transposes are needed.
    # Scale bookkeeping (fp8): U1 carries x64, hbbar the plain y-sum, so
    # ps_q = 2^15*V_true; U2a arrives x2^15 from the host, so
    # vass = 2^15*(V_true + U2a) in one add; the out drain applies 2^-15.
    for rq in range(RB // 4):
        ps_q = ppool.tile([P, 512], fp32, tag="ps")
        for k in range(4):
            r = rq * 4 + k
            for jc in range(JC):
                nc.tensor.matmul(
                    ps_q[k * 32 : k * 32 + B, :D],
                    hbbarTq[:, jc * B : (jc + 1) * B],
                    u1v[:, jc, r, :],
                    start=(jc == 0),
                    stop=(jc == JC - 1),
                    tile_position=(0, k * 32),
                )
        # copy the live PSUM rows (k*32..k*32+B) into the zeroed staging
        # tile at the same (32-aligned) offsets; dead rows stay zero
        vq = dpool.tile([100, D], fp32, tag="vq", bufs=2)
        nc.vector.memset(vq, 0.0)
        for k in range(4):
            nc.vector.tensor_copy(
                out=vq[k * 32 : k * 32 + B, :],
                in_=ps_q[k * 32 : k * 32 + B, :D],
            )
        if STAGE < 4:
            continue
        for ic in range(IC):
            ps_t = ppool.tile([P, 512], fp32, tag="ps")
            nc.tensor.transpose(
                ps_t[:P, :100], vq[:, ic * P : (ic + 1) * P], identity_sq
            )
            # one strided add moves all 4 r's: ps_t cols (k*32 + b),
            # viewed [p, k, b] -> [p, b, k], into vass[:, ic, b, r]
            nc.vector.tensor_tensor(
                out=vass[:, ic, :, rq * 4 : (rq + 1) * 4],
                in0=ps_t[:, :128]
                .rearrange("p (k c) -> p k c", c=32)[:, :, :B]
                .rearrange("p k b -> p b k"),
                in1=u2v[:, ic, None, rq * 4 : (rq + 1) * 4].to_broadcast(
                    (P, B, 4)
                ),
                op=mybir.AluOpType.add,
            )

    if STAGE < 5:
        nc.vector.memset(out_sb[:, :1, :1], 0.0)
        nc.scalar.dma_start(out=out_d.ap(), in_=out_sb)
        return

    # --- out[r, x] per b: contract i; the ACT-engine drain applies the
    # 2^-15 and adds rbbar+bias (per-partition bias AP) in one activation
    for b in range(B):
        ps_o = ppool.tile([P, 512], fp32, tag="ps")
        for ic in range(IC):
            nc.tensor.matmul(
                ps_o[:RB, :S],
                vass[:, ic, b, :],
                hftv[:, b, ic, :],
                start=(ic == 0),
                stop=(ic == IC - 1),
            )
        nc.scalar.activation(
            out_sb[:, b, :],
            ps_o[:RB, :S],
            mybir.ActivationFunctionType.Identity,
            bias=rbbT[:, b : b + 1],
            scale=drain_mult,
        )
    nc.scalar.dma_start(out=out_d.ap(), in_=out_sb)


def _get_nc(n_repeat=1):
    if n_repeat not in _NC_CACHE:
        _NC_CACHE[n_repeat] = _build_nc(n_repeat)
    return _NC_CACHE[n_repeat]


def _np_dts():
    from concourse import mybir

    lodt = mybir.dt.float8e4 if VARIANT == "fp8" else mybir.dt.bfloat16
    return mybir.dt.np(mybir.dt.bfloat16), mybir.dt.np(lodt)


def _prep_inputs(h_forward, h_backward, U_1, U_2, bias):
    bf16, lodt = _np_dts()
    lobytes = np.dtype(lodt).itemsize
    off_u1, off_hb, off_u2, off_bias, tot = _layout(lobytes)
    hf = np.asarray(h_forward, dtype=np.float32)
    hb = np.asarray(h_backward, dtype=np.float32)
    u1 = np.asarray(U_1, dtype=np.float32)
    u2 = np.asarray(U_2, dtype=np.float32)
    bz = np.asarray(bias, dtype=np.float32)

    def u8(a):
        return np.ascontiguousarray(a).view(np.uint8).reshape(P, -1)

    # [i%P, (b, ichunk, x)] bf16
    hft_b = u8(
        hf.transpose(0, 2, 1).reshape(B, IC, P, S).transpose(2, 0, 1, 3).astype(bf16)
    )
    # [j%P, (jchunk, b, y)] fp8/bf16
    hb_b = u8(
        hb.transpose(2, 0, 1).reshape(JC, P, B, S).transpose(1, 0, 2, 3).astype(lodt)
    )

    u1_scale = np.float32(64.0) if VARIANT == "fp8" else np.float32(1.0 / S)
    u2a_scale = np.float32(64.0 * S) if VARIANT == "fp8" else np.float32(1.0)

    in_maps = []
    for c in range(NCORES):
        rs = slice(c * RB, (c + 1) * RB)
        # [j%P, (jchunk, r, i)]
        u1_b = u8(
            (u1[:, rs, :].transpose(2, 1, 0) * u1_scale)
            .reshape(JC, P, RB, D)
            .transpose(1, 0, 2, 3)
            .astype(lodt)
        )
        # [d%P, (dchunk, 2*RB)]: cols 0:RB = U2a*u2a_scale, RB:2RB = U2b/S
        u2_b = u8(
            np.concatenate(
                [
                    u2[:D, rs].reshape(IC, P, RB).transpose(1, 0, 2) * u2a_scale,
                    u2[D:, rs].reshape(IC, P, RB).transpose(1, 0, 2)
                    * np.float32(1.0 / S),
                ],
                axis=2,
            )
        )
        bias_b = u8(np.broadcast_to(bz[rs], (P, RB)))
        mega = np.concatenate([hft_b, u1_b, hb_b, u2_b, bias_b], axis=1)
        assert mega.shape == (P, tot), (mega.shape, tot)
        in_maps.append({"mega": mega})
    return in_maps


def _get_exec():
    """One jitted sharded executable, cached for the process lifetime.

    Repeated kernel() calls reuse it — re-jitting a second executable with
    collectives in the same process has been observed to wedge the NRT
    (NRT_EXEC_UNIT_UNRECOVERABLE), while re-executing one executable is solid.
    """
    if "exec" in _EXEC_CACHE:
        return _EXEC_CACHE["exec"]

    import jax
    from jax.sharding import Mesh, PartitionSpec

    import warnings

    with warnings.catch_warnings():
        warnings.simplefilter("ignore")
        from jax.experimental.shard_map import shard_map

    from concourse import mybir
    from concourse.bass2jax import (
        _bass_exec_p,
        install_neuronx_cc_hook,
        partition_id_tensor,
    )

    install_neuronx_cc_hook()
    nc = _get_nc()
    partition_name = nc.partition_id_tensor.name if nc.partition_id_tensor else None
    in_names, out_names, out_avals = [], [], []
    for alloc in nc.m.functions[0].allocations:
        if not isinstance(alloc, mybir.MemoryLocationSet):
            continue
        name = alloc.memorylocations[0].name
        if alloc.kind == "ExternalInput":
            if name != partition_name:
                in_names.append(name)
        elif alloc.kind == "ExternalOutput":
            out_names.append(name)
            out_avals.append(
                jax.core.ShapedArray(tuple(alloc.tensor_shape), mybir.dt.np(alloc.dtype))
            )
    all_names = in_names + out_names
    if partition_name is not None:
        all_names = all_names + [partition_name]

    def _body(*args):
        operands = list(args)
        if partition_name is not None:
            operands.append(partition_id_tensor())
        return tuple(
            _bass_exec_p.bind(
                *operands,
                out_avals=tuple(out_avals),
                in_names=tuple(all_names),
                out_names=tuple(out_names),
                lowering_input_output_aliases=(),
                sim_require_finite=True,
                sim_require_nnan=True,
                nc=nc,
            )
        )

    devices = jax.devices()[:NCORES]
    mesh = Mesh(np.asarray(devices), ("core",))
    n_args = len(in_names) + len(out_avals)
    fn = jax.jit(
        shard_map(
            _body,
            mesh=mesh,
            in_specs=(PartitionSpec("core"),) * n_args,
            out_specs=(PartitionSpec("core"),) * len(out_names),
            check_rep=False,
        ),
        keep_unused=True,
    )
    sh = jax.sharding.NamedSharding(mesh, PartitionSpec("core"))
    _EXEC_CACHE["exec"] = (fn, sh, in_names, out_names, out_avals)
    return _EXEC_CACHE["exec"]


_EXEC_CACHE = {}


def kernel(h_forward, h_backward, U_1, U_2, bias):
    import jax

    fn, sh, in_names, out_names, out_avals = _get_exec()
    in_maps = _prep_inputs(h_forward, h_backward, U_1, U_2, bias)
    args = [
        jax.device_put(
            np.concatenate([in_maps[c][name] for c in range(NCORES)], axis=0), sh
        )
        for name in in_names
    ]
    for av in out_avals:
        args.append(
            jax.device_put(
                np.zeros((NCORES * av.shape[0], *av.shape[1:]), av.dtype), sh
            )
        )
    out_arrs = fn(*args)
    oi = out_names.index("out")
    full = np.asarray(out_arrs[oi]).reshape(NCORES, RB, B, S)  # [core, RB, B, S]
    out = np.concatenate(list(full), axis=0)  # [R, B, S]
    return np.ascontiguousarray(out.transpose(1, 2, 0))  # [B, S, R]


# revision 13
# speedup vs baseline: 15.1032x; 15.1032x over previous
"""Trainium2 Bass kernel for nn_Biaffine (B=4, S=512, D=512, R=64).

Math: the reference computes
    left = einsum('bxi,irj,byj->bxyr', hf, U1, hb)
    out  = mean_y(left + rf[:, :, None] + rb[:, None] + bias)
The mean over y commutes with everything:
    mean_y(left)[b,x,r] = sum_ij hf[b,x,i] U1[i,r,j] hbbar[b,j],
    hbbar = mean_y(hb).
So out[b,x,r] = sum_i hf[b,x,i] * (V[b,i,r] + U2a[i,r]) + rbbar[b,r] + bias[r]
with V[b,i,r] = sum_j U1[i,r,j] hbbar[b,j], rbbar = hbbar @ U2b.

Sharding: tensor-parallel over r (dep_vec_dim): core c owns r in [8c, 8c+8).
Every core loads all of hb and computes hbbar locally - no collective.
Precision ladder (the 2e-2 rel-err gate allows reduced precision; all
K-dim accumulation stays in fp32 PSUM):
  fp8 variant (default): U1 as e4m3 scaled x64 (hbbar cast to e4m3 too,
  the x64/S undone during the PSUM drain), hb as e4m3, hf as bf16 (the
  rf = hf@U2a term dominates the output so hf stays 16-bit).
  Measured rel-err 8.3e-3. bf16 variant: everything bf16, 3.0e-3.
All tiles are host-prepacked so each DMA is one contiguous run per
partition - strided APs were measured 4.4x slower than packed ones.
"""

import os
import sys

import numpy as np

try:
    import concourse.bass as bass  # noqa: F401
except ImportError:  # pragma: no cover
    sys.path.insert(0, "/opt/trn_rl_repo")

B, S, D, R = 4, 512, 512, 64
NCORES = 8
RB = R // NCORES  # 8 r's per core
P = 128
JC = D // P  # 4 j-chunks
IC = D // P  # 4 i-chunks
SY = S // NCORES  # 64 y's per core (AR variant only)

# per-partition byte offsets inside the single packed input tile: all five
# logical inputs ride one DMA (each extra dma_start costs ~1-2us of in-order
# queue completion latency per body, which dominated v4's tail)
OFF_HFT = 0                       # B*IC*S bf16 = 16384 B
OFF_U1 = OFF_HFT + B * IC * S * 2  # JC*RB*D fp8/bf16
OFF_HB = 0                        # filled in at import below
OFF_U2 = 0
OFF_BIAS = 0
TOT = 0


def _layout(lobytes):
    """Byte offsets for the packed tile; lobytes = 1 (fp8) or 2 (bf16)."""
    off_u1 = OFF_U1
    off_hb = off_u1 + JC * RB * D * lobytes
    off_u2 = off_hb + JC * B * S * lobytes
    off_bias = off_u2 + IC * 2 * RB * 4
    tot = off_bias + RB * 4
    return off_u1, off_hb, off_u2, off_bias, tot

# "fp8": U1 and hb in float8_e4m3 (default). "bf16": U1 and hb in bf16.
VARIANT = os.environ.get("BASS_KERNEL_VARIANT", "fp8")
# truncate the body after stage N (5 = full kernel); timing ablations only
STAGE = int(os.environ.get("BASS_KERNEL_STAGE", "5"))

_NC_CACHE = {}


def _build_nc(n_repeat=1, variant=None):
    import concourse.bacc as bacc
    import concourse.mybir as mybir
    import concourse.tile as tile
    from concourse.masks import make_identity

    if variant is None:
        variant = VARIANT
    fp32 = mybir.dt.float32
    bf16 = mybir.dt.bfloat16
    lodt = mybir.dt.float8e4 if variant == "fp8" else bf16

    nc = bacc.Bacc("TRN2", target_bir_lowering=False, debug=False, num_devices=NCORES)

    u8 = mybir.dt.uint8
    lobytes = 1 if variant == "fp8" else 2
    _, _, _, _, tot = _layout(lobytes)
    mega_d = nc.dram_tensor("mega", [P, tot], u8, kind="ExternalInput")
    out_d = nc.dram_tensor("out", [RB, B, S], fp32, kind="ExternalOutput")

    with tile.TileContext(nc) as tc:
        with (
            tc.tile_pool(name="const", bufs=1) as cpool,
            tc.tile_pool(name="data", bufs=1) as dpool,
            tc.tile_pool(name="psum", bufs=8, space="PSUM") as ppool,
            tc.tile_pool(name="dram", bufs=1, space="DRAM") as drpool,
        ):
            identity_sq = cpool.tile([100, 100], fp32, tag="identity_sq")
            make_identity(nc, identity_sq)
            ones1 = cpool.tile([1, S], fp32, tag="ones1")
            nc.vector.memset(ones1, 1.0)

            for _rep in range(n_repeat):
                _emit_body(
                    nc, dpool, ppool, drpool, fp32, bf16, lodt, ones1,
                    identity_sq, mega_d, out_d, variant,
                )

    nc.compile()
    return nc


def _emit_body(
    nc, dpool, ppool, drpool, fp32, bf16, lodt, ones1, identity_sq,
    mega_d, out_d, variant,
):
    import concourse.mybir as mybir

    u8 = mybir.dt.uint8
    lobytes = 1 if variant == "fp8" else 2
    off_u1, off_hb, off_u2, off_bias, tot = _layout(lobytes)

    # fp8: U1 is scaled x64 on the host (e4m3 underflows below ~2^-9) and
    # 1/S is not folded in; 64*S = 2^15 exactly, undone in the out drain
    drain_mult = 1.0 / (64.0 * S) if variant == "fp8" else 1.0

    hbbarT = dpool.tile([P, JC * B], fp32, tag="hbbarT", bufs=2)
    hbbarTq = dpool.tile([P, JC * B], lodt, tag="hbbarTq", bufs=2)
    vass = dpool.tile([P, IC, B, RB], bf16, tag="vass", bufs=2)
    out_sb = dpool.tile([RB, B, S], fp32, tag="outsb", bufs=2)

    # --- ONE input DMA: all five logical inputs, packed per partition ---
    mega = dpool.tile([P, tot], u8, tag="mega", bufs=2)
    nc.sync.dma_start(out=mega, in_=mega_d.ap())

    # typed views into the packed tile
    hftv = (
        mega[:, OFF_HFT : OFF_HFT + B * IC * S * 2]
        .bitcast(bf16)
        .rearrange("p (b ic x) -> p b ic x", b=B, ic=IC)
    )
    u1v = (
        mega[:, off_u1 : off_u1 + JC * RB * D * lobytes]
        .bitcast(lodt)
        .rearrange("p (jc r i) -> p jc r i", jc=JC, r=RB)
    )
    hbv = (
        mega[:, off_hb : off_hb + JC * B * S * lobytes]
        .bitcast(lodt)
        .rearrange("p (jc b y) -> p jc b y", jc=JC, b=B)
    )
    u2v = (
        mega[:, off_u2 : off_u2 + IC * 2 * RB * 4]
        .bitcast(fp32)
        .rearrange("p (ic c) -> p ic c", ic=IC)
    )
    biasv = mega[0:1, off_bias : off_bias + RB * 4].bitcast(fp32)

    if STAGE < 1:
        nc.vector.memset(out_sb[:, :1, :1], 0.0)
        nc.scalar.dma_start(out=out_d.ap(), in_=out_sb)
        return

    # hbbarT[j, b] = sum_y hb[b, y, j], one 3D-AP reduce per jc
    for jc in range(JC):
        nc.vector.reduce_sum(
            hbbarT[:, jc * B : (jc + 1) * B, None],
            hbv[:, jc, :, :],
            axis=mybir.AxisListType.X,
        )

    # low-precision copy of hbbar for the V matmuls (matches U1's dtype)
    nc.vector.tensor_copy(out=hbbarTq, in_=hbbarT)

    if STAGE < 2:
        nc.vector.memset(out_sb[:, :1, :1], 0.0)
        nc.scalar.dma_start(out=out_d.ap(), in_=out_sb)
        return

    # --- rbbT[r, b] = (hbbar @ U2b + bias)^T computed directly transposed:
    # stationary U2b [j128, RB], moving hbbar [j128, B]; bias via K=1 matmul
    ps_rbt = ppool.tile([P, 512], fp32, tag="ps")
    for jc in range(JC):
        nc.tensor.matmul(
            ps_rbt[:RB, :B],
            u2v[:, jc, RB : 2 * RB],
            hbbarT[:, jc * B : (jc + 1) * B],
            start=(jc == 0),
            stop=False,
        )
    nc.tensor.matmul(
        ps_rbt[:RB, :B], biasv, ones1[:1, :B], start=False, stop=True
    )
    rbbT = dpool.tile([RB, B], fp32, tag="rbbT", bufs=2)
    nc.vector.tensor_copy(out=rbbT, in_=ps_rbt[:RB, :B])

    if STAGE < 3:
        nc.vector.memset(out_sb[:, :1, :1], 0.0)
        nc.scalar.dma_start(out=out_d.ap(), in_=out_sb)
        return

    # --- V[i, r, b] via stationary-U1 matmuls: lhsT = U1 block [j128, i128]
    # (fp8 -> fast-weight-load), rhs = hbbarTq [j128, B]. V lands in PSUM
    # already i-major - no PE transposes, no PSUM->SBUF->PE round trip.
    # All 8r x 4ic blocks share one PSUM bank (disjoint 4-col groups).
    # Scale bookkeeping (fp8): U1 carries x64, hbbar the plain y-sum, so
    # ps_v = 2^15*V_true; U2a arrives x2^15 from the host, so
    # vass = 2^15*(V_true + U2a) in one add; the out drain applies 2^-15.
    ps_v = ppool.tile([P, 512], fp32, tag="ps")
    for ic in range(IC):
        for r in range(RB):
            for jc in range(JC):
                nc.tensor.matmul(
                    ps_v[:, ic * 128 + r * B : ic * 128 + (r + 1) * B],
                    u1v[:, jc, r, ic * P : (ic + 1) * P],
                    hbbarTq[:, jc * B : (jc + 1) * B],
                    start=(jc == 0),
                    stop=(jc == JC - 1),
                )
        if STAGE < 4:
            continue
        # drain+bias: vass[:, ic, b, r] = ps_v[p, (r, b)] + U2a[p, ic, r]
        nc.vector.tensor_tensor(
            out=vass[:, ic, :, :],
            in0=ps_v[:, ic * 128 : ic * 128 + RB * B]
            .rearrange("p (r b) -> p r b", b=B)
            .rearrange("p r b -> p b r"),
            in1=u2v[:, ic, None, :RB].to_broadcast((P, B, RB)),
            op=mybir.AluOpType.add,
        )

    if STAGE < 5:
        nc.vector.memset(out_sb[:, :1, :1], 0.0)
        nc.scalar.dma_start(out=out_d.ap(), in_=out_sb)
        return

    # --- out[r, x] per b: contract i; the ACT-engine drain applies the
    # 2^-15 and adds rbbar+bias (per-partition bias AP) in one activation
    for b in range(B):
        ps_o = ppool.tile([P, 512], fp32, tag="ps")
        for ic in range(IC):
            nc.tensor.matmul(
                ps_o[:RB, :S],
                vass[:, ic, b, :],
                hftv[:, b, ic, :],
                start=(ic == 0),
                stop=(ic == IC - 1),
            )
        nc.scalar.activation(
            out_sb[:, b, :],
            ps_o[:RB, :S],
            mybir.ActivationFunctionType.Identity,
            bias=rbbT[:, b : b + 1],
            scale=drain_mult,
        )
    nc.scalar.dma_start(out=out_d.ap(), in_=out_sb)


def _get_nc(n_repeat=1):
    if n_repeat not in _NC_CACHE:
        _NC_CACHE[n_repeat] = _build_nc(n_repeat)
    return _NC_CACHE[n_repeat]


def _np_dts():
    from concourse import mybir

    lodt = mybir.dt.float8e4 if VARIANT == "fp8" else mybir.dt.bfloat16
    return mybir.dt.np(mybir.dt.bfloat16), mybir.dt.np(lodt)


def _prep_inputs(h_forward, h_backward, U_1, U_2, bias):
    bf16, lodt = _np_dts()
    lobytes = np.dtype(lodt).itemsize
    off_u1, off_hb, off_u2, off_bias, tot = _layout(lobytes)
    hf = np.asarray(h_forward, dtype=np.float32)
    hb = np.asarray(h_backward, dtype=np.float32)
    u1 = np.asarray(U_1, dtype=np.float32)
    u2 = np.asarray(U_2, dtype=np.float32)
    bz = np.asarray(bias, dtype=np.float32)

    def u8(a):
        return np.ascontiguousarray(a).view(np.uint8).reshape(P, -1)

    # [i%P, (b, ichunk, x)] bf16
    hft_b = u8(
        hf.transpose(0, 2, 1).reshape(B, IC, P, S).transpose(2, 0, 1, 3).astype(bf16)
    )
    # [j%P, (jchunk, b, y)] fp8/bf16
    hb_b = u8(
        hb.transpose(2, 0, 1).reshape(JC, P, B, S).transpose(1, 0, 2, 3).astype(lodt)
    )

    u1_scale = np.float32(64.0) if VARIANT == "fp8" else np.float32(1.0 / S)
    u2a_scale = np.float32(64.0 * S) if VARIANT == "fp8" else np.float32(1.0)

    in_maps = []
    for c in range(NCORES):
        rs = slice(c * RB, (c + 1) * RB)
        # [j%P, (jchunk, r, i)]
        u1_b = u8(
            (u1[:, rs, :].transpose(2, 1, 0) * u1_scale)
            .reshape(JC, P, RB, D)
            .transpose(1, 0, 2, 3)
            .astype(lodt)
        )
        # [d%P, (dchunk, 2*RB)]: cols 0:RB = U2a*u2a_scale, RB:2RB = U2b/S
        u2_b = u8(
            np.concatenate(
                [
                    u2[:D, rs].reshape(IC, P, RB).transpose(1, 0, 2) * u2a_scale,
                    u2[D:, rs].reshape(IC, P, RB).transpose(1, 0, 2)
                    * np.float32(1.0 / S),
                ],
                axis=2,
            )
        )
        bias_b = u8(np.broadcast_to(bz[rs], (P, RB)))
        mega = np.concatenate([hft_b, u1_b, hb_b, u2_b, bias_b], axis=1)
        assert mega.shape == (P, tot), (mega.shape, tot)
        in_maps.append({"mega": mega})
    return in_maps


def _get_exec():
    """One jitted sharded executable, cached for the process lifetime.

    Repeated kernel() calls reuse it — re-jitting a second executable with
    collectives in the same process has been observed to wedge the NRT
    (NRT_EXEC_UNIT_UNRECOVERABLE), while re-executing one executable is solid.
    """
    if "exec" in _EXEC_CACHE:
        return _EXEC_CACHE["exec"]

    import jax
    from jax.sharding import Mesh, PartitionSpec

    import warnings

    with warnings.catch_warnings():
        warnings.simplefilter("ignore")
        from jax.experimental.shard_map import shard_map

    from concourse import mybir
    from concourse.bass2jax import (
        _bass_exec_p,
        install_neuronx_cc_hook,
        partition_id_tensor,
    )

    install_neuronx_cc_hook()
    nc = _get_nc()
    partition_name = nc.partition_id_tensor.name if nc.partition_id_tensor else None
    in_names, out_names, out_avals = [], [], []
    for alloc in nc.m.functions[0].allocations:
        if not isinstance(alloc, mybir.MemoryLocationSet):
            continue
        name = alloc.memorylocations[0].name
        if alloc.kind == "ExternalInput":
            if name != partition_name:
                in_names.append(name)
        elif alloc.kind == "ExternalOutput":
            out_names.append(name)
            out_avals.append(
                jax.core.ShapedArray(tuple(alloc.tensor_shape), mybir.dt.np(alloc.dtype))
            )
    all_names = in_names + out_names
    if partition_name is not None:
        all_names = all_names + [partition_name]

    def _body(*args):
        operands = list(args)
        if partition_name is not None:
            operands.append(partition_id_tensor())
        return tuple(
            _bass_exec_p.bind(
                *operands,
                out_avals=tuple(out_avals),
                in_names=tuple(all_names),
                out_names=tuple(out_names),
                lowering_input_output_aliases=(),
                sim_require_finite=True,
                sim_require_nnan=True,
                nc=nc,
            )
        )

    devices = jax.devices()[:NCORES]
    mesh = Mesh(np.asarray(devices), ("core",))
    n_args = len(in_names) + len(out_avals)
    fn = jax.jit(
        shard_map(
            _body,
            mesh=mesh,
            in_specs=(PartitionSpec("core"),) * n_args,
            out_specs=(PartitionSpec("core"),) * len(out_names),
            check_rep=False,
        ),
        keep_unused=True,
    )
    sh = jax.sharding.NamedSharding(mesh, PartitionSpec("core"))
    _EXEC_CACHE["exec"] = (fn, sh, in_names, out_names, out_avals)
    return _EXEC_CACHE["exec"]


_EXEC_CACHE = {}


def kernel(h_forward, h_backward, U_1, U_2, bias):
    import jax

    fn, sh, in_names, out_names, out_avals = _get_exec()
    in_maps = _prep_inputs(h_forward, h_backward, U_1, U_2, bias)
    args = [
        jax.device_put(
            np.concatenate([in_maps[c][name] for c in range(NCORES)], axis=0), sh
        )
        for name in in_names
    ]
    for av in out_avals:
        args.append(
            jax.device_put(
                np.zeros((NCORES * av.shape[0], *av.shape[1:]), av.dtype), sh
            )
        )
    out_arrs = fn(*args)
    oi = out_names.index("out")
    full = np.asarray(out_arrs[oi]).reshape(NCORES, RB, B, S)  # [core, RB, B, S]
    out = np.concatenate(list(full), axis=0)  # [R, B, S]
    return np.ascontiguousarray(out.transpose(1, 2, 0))  # [B, S, R]
